# revision 22
# baseline (speedup 1.0000x reference)
"""DAG-LSTM Trainium2 kernel.

Strategy
--------
The reference scans 128 LSTM "nodes" whose inputs are means of predecessor
states, and returns ONLY the last node's hidden state.  Because the DAG
structure (preds / pred_counts) arrives as concrete host data, the Bass
program is specialized on it at build time:

* dead-code elimination: only nodes transitively feeding node N-1 are
  computed (24 of 128 for the shipped DAG),
* all live h/c states stay resident in SBUF (bf16) -- zero HBM state
  traffic,
* the 1/cnt mean scaling of h is folded into pre-scaled copies of w_hh
  (stationary operand), and into the scalar of a fused
  scalar_tensor_tensor op on the c path,
* gate bias rides the ACT activation instruction's per-partition bias
  operand for free.

Sharding: data-parallel over batch across the 8 NeuronCores (B=2048 ->
256 rows/core); weights + DAG structure replicated.  Layout on chip is
[H(part), B(free)] so the recurrent matmul needs no transposes:
gates[4H, B] = w_ihT.T @ xT + w_hhT.T @ h_in, accumulated in PSUM (fp32).
"""

import os
import sys
from collections import Counter

import numpy as np


def _ensure_import_paths():
    try:
        import concourse.bass  # noqa: F401
        return
    except Exception:
        pass
    for p in ("/opt/trn_rl_repo", "/root/.axon_site/_ro/trn_rl_repo"):
        if os.path.isdir(p) and p not in sys.path:
            sys.path.insert(0, p)
    import concourse.bass  # noqa: F401


# ---------------------------------------------------------------------------
# DAG analysis (host side, uses the concrete preds / pred_counts values)
# ---------------------------------------------------------------------------

def _analyze_dag(preds, counts):
    """Return (order, info) for the transitively-needed nodes.

    order: needed node indices ascending (valid topo order: preds < node).
    info[j] = (cnt, items) where items = sorted list of (slot, multiplicity)
    over the distinct predecessor buffer slots of node j.
    """
    n_nodes = preds.shape[0]
    needed = {n_nodes - 1}
    stack = [n_nodes - 1]
    while stack:
        j = stack.pop()
        cnt = int(counts[j, 0])
        for s in preds[j, :cnt]:
            s = int(s)
            if s >= 1 and (s - 1) not in needed:
                needed.add(s - 1)
                stack.append(s - 1)
    order = sorted(needed)
    info = {}
    for j in order:
        cnt = int(counts[j, 0])
        items = sorted(Counter(int(s) for s in preds[j, :cnt]).items())
        info[j] = (cnt, items)
    return order, info


def _slot_liveness(order, info, n_nodes):
    """last_use[slot] = last node index that reads the slot (-1 if never)."""
    last_use = {0: -1}
    for j in order:
        last_use[j + 1] = -1
    for j in order:
        _, items = info[j]
        for s, _m in items:
            last_use[s] = max(last_use[s], j)
    # the final slot is the output; keep it alive to the end
    last_use[n_nodes] = n_nodes + 1
    return last_use


# ---------------------------------------------------------------------------
# Bass program builder
# ---------------------------------------------------------------------------

def _build_program(order, info, n_nodes, scales, node_scale_var):
    """Build the Bass module (single-core program; SPMD over batch shards)."""
    import concourse.bacc as bacc
    import concourse.mybir as mybir
    from concourse.tile import TileContext, add_dep_helper
    from contextlib import ExitStack

    dt = mybir.dt
    AF = mybir.ActivationFunctionType
    ALU = mybir.AluOpType
    n = len(order)
    nvar = len(scales)

    nc = bacc.Bacc()
    x_d = nc.dram_tensor("x", [n, 128, 512], dt.bfloat16, kind="ExternalInput")
    wih_d = nc.dram_tensor("wih", [2, 128, 1024], dt.bfloat16, kind="ExternalInput")
    whh_d = nc.dram_tensor("whh", [nvar, 2, 128, 1024], dt.bfloat16, kind="ExternalInput")
    bias_d = nc.dram_tensor("bias", [128, 8], dt.float32, kind="ExternalInput")
    h0_d = nc.dram_tensor("h0", [128, 512], dt.bfloat16, kind="ExternalInput")
    c0_d = nc.dram_tensor("c0", [128, 512], dt.bfloat16, kind="ExternalInput")
    out_d = nc.dram_tensor("out", [128, 512], dt.float32, kind="ExternalOutput")

    # SBUF budget (KB/partition): states 2/slot, x 1/node, weights 2/tile.
    budget_kb = (n + 1) * 2 + n * 3 + (2 + 2 * nvar) * 2 + 40
    unique_bufs = budget_kb < 200
    if unique_bufs:
        sched = {0: 0}
        for j in order:
            sched[j + 1] = j + 1
    else:
        # physical state-slot assignment (free-list over liveness)
        last_use = _slot_liveness(order, info, n_nodes)
        phys_of = {}
        free = []
        next_phys = 0

        def alloc_phys():
            nonlocal next_phys
            if free:
                return free.pop()
            p = next_phys
            next_phys += 1
            return p

        phys_of[0] = alloc_phys()
        sched = {0: phys_of[0]}
        for j in order:
            for s in sorted(phys_of):
                lu = last_use.get(s, -1)
                if lu < j and lu != n_nodes + 1:
                    free.append(phys_of.pop(s))
            free.sort(reverse=True)
            p = alloc_phys()
            phys_of[j + 1] = p
            sched[j + 1] = p

    gate_funcs = [AF.Sigmoid, AF.Sigmoid, AF.Tanh, AF.Sigmoid]  # i, f, g, o

    with TileContext(nc) as tc, ExitStack() as ctx:
        wpool = ctx.enter_context(tc.tile_pool(name="weights", bufs=1))
        spool = ctx.enter_context(tc.tile_pool(name="state", bufs=1))
        xpool = ctx.enter_context(tc.tile_pool(name="xin", bufs=4))
        gpool = ctx.enter_context(tc.tile_pool(name="gates", bufs=2))
        pspool = ctx.enter_context(tc.tile_pool(name="psum", bufs=2, space="PSUM"))

        wih_sb = []
        for k in range(2):
            wt = wpool.tile([128, 1024], dt.bfloat16, name=f"wih{k}", tag=f"wih{k}")
            nc.sync.dma_start(out=wt[:, :], in_=wih_d[k, :, :])
            wih_sb.append(wt)
        whh_sb = []
        for v in range(nvar):
            row = []
            for k in range(2):
                wt = wpool.tile([128, 1024], dt.bfloat16, name=f"whh{v}_{k}", tag=f"whh{v}_{k}")
                nc.sync.dma_start(out=wt[:, :], in_=whh_d[v, k, :, :])
                row.append(wt)
            whh_sb.append(row)
        bias_sb = wpool.tile([128, 8], dt.float32, name="biassb", tag="biassb")
        nc.sync.dma_start(out=bias_sb[:, :], in_=bias_d[:, :])

        # Warmups: absorb the const-DMA queue waits (and the ACT
        # table-set load) into dedicated low-dependency instructions so
        # steady-state instructions stay within the HW wait-slot limit.
        warm_act = gpool.tile([128, 8], dt.float32, name="warmact", tag="warmact", bufs=1)
        nc.scalar.activation(warm_act[:, 0:8], bias_sb[:, 0:8],
                             AF.Sigmoid, bias=bias_sb[:, 0:1])
        warm_pairs = [(wih_sb[0], wih_sb[1])]
        for v in range(nvar):
            warm_pairs.append((whh_sb[v][0], whh_sb[v][1]))
        for wi_, (a, b) in enumerate(warm_pairs):
            warm_ps = pspool.tile([128, 16], dt.float32, name=f"warmps{wi_}",
                                  tag=f"ps{wi_ % 4}")
            nc.tensor.matmul(warm_ps[:, 0:16], a[:, 0:128], b[:, 0:16],
                             start=True, stop=True)

        # state tiles: one (h, c) pair of [128, 512] bf16 tiles per slot
        def new_state_tiles(slot):
            p = sched[slot]
            ht = spool.tile([128, 512], dt.bfloat16, name=f"h_s{slot}", tag=f"hP{p}")
            ct = spool.tile([128, 512], dt.bfloat16, name=f"c_s{slot}", tag=f"cP{p}")
            return ht, ct

        h_t = {}
        c_t = {}
        h_t[0], c_t[0] = new_state_tiles(0)
        nc.sync.dma_start(out=h_t[0][:, :], in_=h0_d[:, :])
        nc.sync.dma_start(out=c_t[0][:, :], in_=c0_d[:, :])
        # Absorb the h0/c0 DMA-queue waits into single-dependency DVE ops
        # (keeps steady-state instructions at <=1 wait; Bacc splits the rest).
        absorb = gpool.tile([128, 2], dt.float32, name="absorb", tag="absorb", bufs=1)
        nc.vector.tensor_copy(absorb[:, 0:1], h_t[0][:, 0:1])
        nc.vector.tensor_copy(absorb[:, 1:2], c_t[0][:, 0:1])

        for idx, j in enumerate(order):
            cnt, items = info[j]
            is_final = j == n_nodes - 1

            x_tag = f"x{j}" if unique_bufs else "x"
            xt = xpool.tile([128, 512], dt.bfloat16, name=f"x_{j}", tag=x_tag,
                            bufs=1 if unique_bufs else None)
            nc.sync.dma_start(out=xt[:, :], in_=x_d[idx, :, :])

            # --- predecessor aggregation -------------------------------
            # h side: matmul rhs is the (multiplicity-weighted) SUM of
            # predecessor h tiles; the 1/cnt lands in the w_hh variant.
            # c side: weighted sum; 1/cnt rides the STT scalar in u1.
            if len(items) == 1:
                s0, _m0 = items[0]
                h_rhs = h_t[s0]
                c_src = c_t[s0]
                c_pre = float(_m0) / float(cnt)
            else:
                sum_tag = f"sum{j}" if unique_bufs else "sum"
                h_sum = gpool.tile([128, 512], dt.bfloat16, name=f"hsum_{j}",
                                   tag=f"h{sum_tag}", bufs=1 if unique_bufs else None)
                c_sum = gpool.tile([128, 512], dt.bfloat16, name=f"csum_{j}",
                                   tag=f"c{sum_tag}", bufs=1 if unique_bufs else None)
                for dst, src_t in ((h_sum, h_t), (c_sum, c_t)):
                    (s0, m0), (s1, m1) = items[0], items[1]
                    rest = []
                    if m1 > 1:
                        rest.extend([s1] * (m1 - 1))
                    for s, m in items[2:]:
                        rest.extend([s] * m)
                    if m0 > 1:
                        nc.vector.scalar_tensor_tensor(
                            dst[:, :], src_t[s0][:, :], float(m0), src_t[s1][:, :],
                            ALU.mult, ALU.add,
                        )
                    else:
                        nc.vector.tensor_add(dst[:, :], src_t[s0][:, :], src_t[s1][:, :])
                    for s in rest:
                        nc.vector.tensor_add(dst[:, :], dst[:, :], src_t[s][:, :])
                h_rhs = h_sum
                c_src = c_sum
                c_pre = 1.0 / float(cnt)

            wv = node_scale_var[j]

            # --- gates: 4 PSUM tiles (i,f,g,o), halves at cols kh*256 --
            ps = []
            for X in range(4):
                pt = pspool.tile([128, 512], dt.float32, name=f"ps{j}_{X}", tag=f"ps{X}")
                ps.append(pt)
            # One accumulation group per PSUM tile: start=True zeroes the
            # bank on the tile's first MM; stop=True closes it on the last.
            # ih matmuls first (independent of the recurrent chain)
            for X in range(4):
                for kh in range(2):
                    m = 2 * X + kh
                    o_ap = ps[X][:, kh * 256:(kh + 1) * 256]
                    nc.tensor.matmul(o_ap, wih_sb[0][:, m * 128:(m + 1) * 128],
                                     xt[:, 0:256], start=(kh == 0), stop=False)
                    nc.tensor.matmul(o_ap, wih_sb[1][:, m * 128:(m + 1) * 128],
                                     xt[:, 256:512], start=False, stop=False)
            # recurrent matmuls
            for X in range(4):
                for kh in range(2):
                    m = 2 * X + kh
                    o_ap = ps[X][:, kh * 256:(kh + 1) * 256]
                    nc.tensor.matmul(o_ap, whh_sb[wv][0][:, m * 128:(m + 1) * 128],
                                     h_rhs[:, 0:256], start=False, stop=False)
                    nc.tensor.matmul(o_ap, whh_sb[wv][1][:, m * 128:(m + 1) * 128],
                                     h_rhs[:, 256:512], start=False, stop=(kh == 1))

            # --- activations + cell update -----------------------------
            gt = []
            for X in range(4):
                t = gpool.tile([128, 512], dt.bfloat16, name=f"g{X}_{j}", tag=f"g{X}")
                for kh in range(2):
                    m = 2 * X + kh
                    nc.scalar.activation(
                        t[:, kh * 256:(kh + 1) * 256],
                        ps[X][:, kh * 256:(kh + 1) * 256],
                        gate_funcs[X],
                        bias=bias_sb[:, m:m + 1],
                    )
                gt.append(t)
            s_i, s_f, t_g, s_o = gt

            h_new, c_new = new_state_tiles(j + 1)
            fin = "F" if is_final else ""
            u2 = gpool.tile([128, 512], dt.bfloat16, name=f"u2_{j}", tag=f"u2{fin}",
                            bufs=1 if is_final else None)
            i_u2 = nc.vector.tensor_mul(u2[:, :], s_i[:, :], t_g[:, :])
            u1 = gpool.tile([128, 512], dt.bfloat16, name=f"u1_{j}", tag=f"u1{fin}",
                            bufs=1 if is_final else None)
            i_u1 = nc.vector.scalar_tensor_tensor(
                u1[:, :], c_src[:, :], c_pre, s_f[:, :], ALU.mult, ALU.mult,
            )
            # keep u1 after u2 in the DVE stream so u1's s_f (ACT) dep is
            # already covered by u2's later-tick ACT wait (1-wait limit)
            add_dep_helper(i_u1.ins, i_u2.ins, sync=False,
                           reason="share ACT wait between u2 and u1")
            nc.vector.tensor_add(c_new[:, :], u1[:, :], u2[:, :])
            t_c = gpool.tile([128, 512], dt.bfloat16, name=f"tc_{j}", tag=f"tc{fin}",
                             bufs=1 if is_final else None)
            nc.scalar.activation(t_c[:, :], c_new[:, :], AF.Tanh)
            if is_final:
                out_sb = gpool.tile([128, 512], dt.float32, name="outsb", tag="outsb")
                nc.vector.tensor_mul(out_sb[:, :], s_o[:, :], t_c[:, :])
                nc.gpsimd.dma_start(out=out_d[:, :], in_=out_sb[:, :])
            else:
                nc.vector.tensor_mul(h_new[:, :], s_o[:, :], t_c[:, :])
            h_t[j + 1] = h_new
            c_t[j + 1] = c_new

    nc.finalize()
    return nc


# ---------------------------------------------------------------------------
# Host-side entry point
# ---------------------------------------------------------------------------

_N_CORES = 8


def _prep_inputs(dags, h0, c0, w_ih, w_hh, b_ih, b_hh, order, scales):
    import ml_dtypes
    bf16 = ml_dtypes.bfloat16

    B = dags.shape[0]
    n = len(order)
    b_l = B // _N_CORES

    wihT = np.ascontiguousarray(w_ih.T.reshape(2, 128, 1024)).astype(bf16)
    whhT = w_hh.T.reshape(2, 128, 1024)
    whh_vars = np.stack([whhT * np.float32(s) for s in scales]).astype(bf16)
    bias = (b_ih + b_hh).astype(np.float32)
    bias_sb = np.ascontiguousarray(bias.reshape(8, 128).T)

    in_maps = []
    for c in range(_N_CORES):
        bsl = slice(c * b_l, (c + 1) * b_l)
        d = dags[bsl][:, order, :]                      # [b_l, n, IN]
        dtr = np.ascontiguousarray(d.transpose(1, 2, 0))  # [n, IN, b_l]
        xsb = np.ascontiguousarray(
            dtr.reshape(n, 2, 128, 256).transpose(0, 2, 1, 3).reshape(n, 128, 512)
        ).astype(bf16)

        def state_layout(a):  # [b_l, H] -> [128, 512]
            return np.ascontiguousarray(
                a.T.reshape(2, 128, 256).transpose(1, 0, 2).reshape(128, 512)
            )

        h0sb = state_layout(h0[0, bsl]).astype(bf16)
        c0sb = state_layout(c0[0, bsl]).astype(bf16)
        in_maps.append({
            "x": xsb, "wih": wihT, "whh": whh_vars,
            "bias": bias_sb, "h0": h0sb, "c0": c0sb,
        })
    return in_maps


def _run(dags, h0, c0, w_ih, w_hh, b_ih, b_hh, preds, pred_counts, trace=False):
    _ensure_import_paths()
    from concourse.bass_utils import run_bass_kernel_spmd

    dags = np.asarray(dags, dtype=np.float32)
    h0 = np.asarray(h0, dtype=np.float32)
    c0 = np.asarray(c0, dtype=np.float32)
    w_ih = np.asarray(w_ih, dtype=np.float32)
    w_hh = np.asarray(w_hh, dtype=np.float32)
    b_ih = np.asarray(b_ih, dtype=np.float32)
    b_hh = np.asarray(b_hh, dtype=np.float32)
    preds = np.asarray(preds)
    pred_counts = np.asarray(pred_counts)

    B, n_nodes, _ = dags.shape
    H = h0.shape[2]
    assert B % _N_CORES == 0

    order, info = _analyze_dag(preds, pred_counts)

    # weight-scale variants (h-mean folded into stationary weights)
    scale_list = []
    node_scale_var = {}
    for j in order:
        cnt, items = info[j]
        s = (float(items[0][1]) / cnt) if len(items) == 1 else (1.0 / cnt)
        s = np.float32(s)
        if s not in scale_list:
            scale_list.append(s)
        node_scale_var[j] = scale_list.index(s)

    nc = _build_program(order, info, n_nodes, scale_list, node_scale_var)
    in_maps = _prep_inputs(dags, h0, c0, w_ih, w_hh, b_ih, b_hh, order, scale_list)

    res = run_bass_kernel_spmd(nc, in_maps, core_ids=list(range(_N_CORES)),
                               trace=trace)

    b_l = B // _N_CORES
    out = np.empty((B, H), dtype=np.float32)
    for c in range(_N_CORES):
        o = np.asarray(res.results[c]["out"])           # [128, 512]
        out[c * b_l:(c + 1) * b_l] = (
            o.reshape(128, 2, 256).transpose(2, 1, 0).reshape(256, 256)
        )
    return out, res


def kernel(dags, h0, c0, w_ih, w_hh, b_ih, b_hh, preds, pred_counts):
    out, _ = _run(dags, h0, c0, w_ih, w_hh, b_ih, b_hh, preds, pred_counts)
    return out
